# revision 35
# baseline (speedup 1.0000x reference)
"""GroupedQueryAttention TRN2 kernel (v2).

Sharding: 8 cores = (batch b in 0..1) x (kv-group g in 0..3). Each core
computes, for its batch and its kv head group (1 kv head, 4 query heads):
  q = x[b] @ Wq[:, g*256:(g+1)*256]          [2048, 256]
  k = x[b] @ Wkv[:, g*64:(g+1)*64]           [2048, 64]
  v = x[b] @ Wkv[:, 256+g*64:256+(g+1)*64]   [2048, 64]
  causal softmax attention per head          [2048, 256]
  partial_out = attn_out @ Wo[g*256:(g+1)*256, :]   [2048, 1024]
Host sums the 4 partials per batch (row-parallel Wo).

All operands fp16 (same precision class as fp32r, half the weight-load /
DVE / DMA cost). On-chip layout is fully transposed (feature dims on
partitions):
  - scores are computed as S^T[tk, tq]; heads are processed in parity
    pairs (even head kT/qT at partitions 0:64, odd at 64:128) so the two
    64-row score matmuls land on different PE row groups and overlap.
  - both heads' scores for one key tile share a psum tile (even slab at
    bank 0, odd at bank 1) -> usually one exp per key tile.
  - causal masking: exp everything, then zero above-diagonal
    probabilities with a 0/1 fp16 mask multiply on the (otherwise idle)
    Pool engine.
  - softmax denominators come from ones-columns appended to v. The
    normalization reshapes l to partition-major via a DRAM bounce (a
    [1,1024] single-lane DVE reciprocal costs ~6.5us on HW; [128,8] is
    ~60ns), reciprocals there, and partition-broadcasts 1/l back from
    DRAM; queries stay on the free axis throughout (no transposes).
    The very last group uses PE transposes instead of the DRAM bounce
    (the PE is idle on the tail and DMA latency is on the critical path).
  - odd-parity avT slabs reach avt partitions 64:128 via SBUF->SBUF DMA;
    kT for the odd parity is duplicated to partitions 64:128 by DMA.
  - DMA issue costs ~600ns on the issuing engine; startup/tail issues
    are split between the two HWDGE engines (SP + ACT).
  - out-projection of chunk c-1 is emitted after both head pairs of
    chunk c so its PE work covers the DMA-latency-bound normalize chain.
"""

import numpy as np
import ml_dtypes

import concourse.bass as bass
import concourse.mybir as mybir
import concourse.tile as tile
from concourse import bacc
from concourse.bass_utils import run_bass_kernel_spmd

B, T, DIM = 2, 2048, 1024
NH, NKV = 16, 4
HD = DIM // NH  # 64
R = NH // NKV  # 4
HQ = R * HD  # 256 query cols per core
NJ = T // 128  # 16 key tiles
NCH = T // 512  # 4 query chunks of 512

F16 = mybir.dt.float16
F32 = mybir.dt.float32
F32R = mybir.dt.float32r
F8 = mybir.dt.float8e4

_CACHED_NC = None


def _cfg(c, j):
    """Per (chunk, key-tile): (tq start within chunk, width)."""
    m = j - 4 * c
    if m < 0:
        return 0, 512
    return 128 * m, 512 - 128 * m


def build_nc():
    nc = bacc.Bacc()
    xT = nc.declare_dram_parameter("xT", [DIM, T], F16, isOutput=False)
    wq = nc.declare_dram_parameter("wq", [DIM, HQ], F16, isOutput=False)
    wkv = nc.declare_dram_parameter("wkv", [DIM, 128], F16, isOutput=False)
    wo = nc.declare_dram_parameter("wo", [HQ, DIM], F16, isOutput=False)
    ltri = nc.declare_dram_parameter("ltri", [128, 128], F16, isOutput=False)
    idh = nc.declare_dram_parameter("idh", [128, 128], F16, isOutput=False)
    out = nc.declare_dram_parameter("out", [T, DIM], F16, isOutput=True)

    with tile.TileContext(nc) as tc:
        with (
            tc.tile_pool(name="persist", bufs=1) as pp,
            tc.tile_pool(name="vaug_p", bufs=NJ) as vp,
            tc.tile_pool(name="pt_p", bufs=4) as ptp,
            tc.tile_pool(name="nrm_p", bufs=3) as nrp,
            tc.tile_pool(name="out_p", bufs=3) as op,
            tc.tile_pool(name="dram_p", bufs=2, space="DRAM") as dp,
            tc.tile_pool(name="ps_s", bufs=2, space="PSUM") as pss,
            tc.tile_pool(name="ps_av", bufs=2, space="PSUM") as psav,
        ):
            # ---- constants / weights ----
            # DMA issue costs ~600ns each on the issuing engine; split the
            # startup issues between the two HWDGE engines (SP + ACT, which
            # is idle until the first exp) so the first qkv matmul is fed
            # as early as possible.
            wq_sb = pp.tile([128, 8, HQ], F16, tag="wq")
            nc.scalar.dma_start(out=wq_sb, in_=wq.rearrange("(k p) m -> p k m", p=128))
            xt_sb = pp.tile([128, 8, T], F16, tag="xt")
            for kd in range(8):
                deng = nc.sync if kd % 2 == 0 else nc.scalar
                deng.dma_start(
                    out=xt_sb[:, kd, 0:512],
                    in_=xT[kd * 128 : (kd + 1) * 128, 0:512],
                )
            wkv_sb = pp.tile([128, 8, 128], F16, tag="wkv")
            nc.scalar.dma_start(
                out=wkv_sb, in_=wkv.rearrange("(k p) m -> p k m", p=128)
            )
            ident = pp.tile([128, 128], F16, tag="ident")
            nc.sync.dma_start(out=ident, in_=idh[:, :])
            idf32 = pp.tile([1, 1], F32, tag="idf32")
            nc.gpsimd.memset(idf32, 1.0)
            ltri_sb = pp.tile([128, 128], F16, tag="ltri")
            nc.sync.dma_start(out=ltri_sb, in_=ltri[:, :])
            xT_r = xT.rearrange("(k p) t -> p k t", p=128)
            for th in range(1, 4):
                tc_cols = slice(th * 512, (th + 1) * 512)
                deng = nc.sync if th % 2 == 1 else nc.scalar
                deng.dma_start(out=xt_sb[:, :, tc_cols], in_=xT_r[:, :, tc_cols])
            wo_sb = pp.tile([128, 2, DIM], F16, tag="wo")
            nc.sync.dma_start(out=wo_sb, in_=wo.rearrange("(c p) n -> p c n", p=128))

            qt_sb = pp.tile([128, 2, T], F16, tag="qt")  # heads (2h, 2h+1) pairs
            kv_sb = pp.tile([128, T], F16, tag="kv")  # 0:64 kT, 64:128 vT
            kvb_sb = pp.tile([128, T], F16, tag="kvb")  # 64:128 kT dup
            avt01 = pp.tile([128, T], F16, tag="avt01")
            avt23 = pp.tile([128, T], F16, tag="avt23")
            eng = [0]

            def cp(dst, src):
                # alternate drain engine to balance ACT/DVE load
                if eng[0] % 2 == 0:
                    nc.scalar.copy(dst, src)
                else:
                    nc.vector.tensor_copy(out=dst, in_=src)
                eng[0] += 1

            # ---- qkv projections (xT stationary, weights as lhsT) ----
            vaug = [None] * NJ

            def qkv_chunk(n):
                cols = slice(n * 512, (n + 1) * 512)
                for m in range(2):
                    pq = pss.tile([128, 512], F32, tag="s")
                    for kd in range(8):
                        nc.tensor.matmul(
                            pq,
                            lhsT=wq_sb[:, kd, m * 128 : (m + 1) * 128],
                            rhs=xt_sb[:, kd, cols],
                            start=(kd == 0),
                            stop=(kd == 7),
                        )
                    cp(qt_sb[:, m, cols], pq)
                pkv = pss.tile([128, 512], F32, tag="s")
                for kd in range(8):
                    nc.tensor.matmul(
                        pkv,
                        lhsT=wkv_sb[:, kd, :],
                        rhs=xt_sb[:, kd, cols],
                        start=(kd == 0),
                        stop=(kd == 7),
                    )
                cp(kv_sb[0:64, cols], pkv[0:64, :])
                cp(kv_sb[64:128, cols], pkv[64:128, :])
                # kT dup for odd heads via SBUF->SBUF DMA
                nc.sync.dma_start(out=kvb_sb[64:128, cols], in_=kv_sb[0:64, cols])
                for tt in range(4):
                    j = n * 4 + tt
                    ptr = pss.tile([128, 64], F16, tag="s")
                    nc.tensor.transpose(
                        ptr,
                        in_=kv_sb[64:128, j * 128 : (j + 1) * 128],
                        identity=ident[64:128, 64:128],
                    )
                    va = vp.tile([128, 66], F16, tag="vaug")
                    nc.scalar.copy(va[:, 0:64], ptr)
                    nc.gpsimd.memset(va[:, 64:66], 1.0)
                    vaug[j] = va

            # ---- attention: one parity-pair of heads over chunk c ----
            def attn_hp(hp, c, fillers=()):
                """hp in {0,1}: heads (2hp, 2hp+1). Even head at partitions
                0:64, odd at 64:128; their score matmuls overlap on
                different PE row groups. Filler closures (out-proj row
                tiles) are emitted between key tiles so the PE bubbles of
                the ACT-paced attention absorb them."""
                jmax = 4 * c + 3
                avt = avt01 if hp == 0 else avt23
                fillers = list(fillers)
                av = psav.tile([66, 1024], F32, tag="av")
                for j in range(jmax + 1):
                    sa, w = _cfg(c, j)
                    spt = pss.tile([128, 1024], F32, tag="s")
                    ptt = ptp.tile([128, 1024], F16, tag="pt")
                    jc = slice(j * 128, (j + 1) * 128)
                    qc = slice(c * 512 + sa, c * 512 + sa + w)
                    # even head at psum cols 0:w (bank 0), odd head at
                    # 512:512+w (bank 1) — matmul outputs may not cross a
                    # psum bank boundary.
                    nc.tensor.matmul(
                        spt[:, 0:w],
                        lhsT=kv_sb[0:64, jc],
                        rhs=qt_sb[0:64, hp, qc],
                        start=True,
                        stop=True,
                    )
                    nc.tensor.matmul(
                        spt[:, 512 : 512 + w],
                        lhsT=kvb_sb[64:128, jc],
                        rhs=qt_sb[64:128, hp, qc],
                        start=True,
                        stop=True,
                    )
                    if w == 512:
                        nc.scalar.activation(
                            out=ptt[:, 0:1024],
                            in_=spt[:, 0:1024],
                            func=mybir.ActivationFunctionType.Exp,
                            scale=0.125,
                        )
                    else:
                        nc.scalar.activation(
                            out=ptt[:, 0:w],
                            in_=spt[:, 0:w],
                            func=mybir.ActivationFunctionType.Exp,
                            scale=0.125,
                        )
                        nc.scalar.activation(
                            out=ptt[:, 512 : 512 + w],
                            in_=spt[:, 512 : 512 + w],
                            func=mybir.ActivationFunctionType.Exp,
                            scale=0.125,
                        )
                    if j >= 4 * c:
                        # zero above-diagonal probs (tile-local cols 0:128
                        # of each head's slab)
                        nc.gpsimd.tensor_mul(
                            out=ptt[:, 0:128], in0=ptt[:, 0:128], in1=ltri_sb
                        )
                        nc.gpsimd.tensor_mul(
                            out=ptt[:, 512:640],
                            in0=ptt[:, 512:640],
                            in1=ltri_sb,
                        )
                    nc.tensor.matmul(
                        av[:, sa : sa + w],
                        lhsT=vaug[j][:, 0:66],
                        rhs=ptt[:, 0:w],
                        start=(j == 0),
                        stop=(j == jmax),
                    )
                    nc.tensor.matmul(
                        av[:, 512 + sa : 512 + sa + w],
                        lhsT=vaug[j][:, 0:66],
                        rhs=ptt[:, 512 : 512 + w],
                        start=(j == 0),
                        stop=(j == jmax),
                    )
                    if fillers:
                        fillers.pop(0)()
                while fillers:
                    fillers.pop(0)()
                # normalize: avT[:, q] /= l[q]  (queries on the free axis).
                # A [1,1024] single-lane DVE reciprocal costs ~6.5us on HW, so
                # reshape l to partition-major (via a DRAM bounce — SBUF APs
                # cannot exchange partition/free dims), reciprocal there
                # (~60ns), then bounce back and partition-broadcast from DRAM.
                ccols = slice(c * 512, (c + 1) * 512)
                rt = nrp.tile([128, 8], F16, tag="rt")
                if hp == 1 and c == NCH - 1:
                    # tail group: the l->column reshape sits on the critical
                    # path with an otherwise-idle PE; transpose on the PE
                    # instead of the DRAM round trip (saves ~4us of latency).
                    # Strided fp32 slices keep the q = p*8+i layout of the
                    # DMA path and 4-byte weight/psum alignment.
                    lrow32 = nrp.tile([1, 1024], F32, tag="lrow32")
                    nc.vector.tensor_copy(out=lrow32, in_=av[64:65, :])
                    lrow_r = lrow32.rearrange("o (p i) -> o p i", p=128)
                    ptl = pss.tile([128, 8], F32, tag="s")
                    for i in range(8):
                        nc.tensor.transpose(
                            ptl[:, i : i + 1],
                            in_=lrow_r[0:1, :, i],
                            identity=idf32,
                        )
                    with nc.allow_low_precision(reason="1/l fits fp16"):
                        nc.vector.reciprocal(out=rt, in_=ptl)
                else:
                    lrow = nrp.tile([1, 1024], F16, tag="lrow")
                    with nc.allow_low_precision(reason="l fits fp16"):
                        nc.vector.tensor_copy(out=lrow, in_=av[64:65, :])
                    rowd = dp.tile([128, 8], F16, tag="rowd")
                    nc.sync.dma_start(out=rowd, in_=lrow)
                    lcol = nrp.tile([128, 8], F16, tag="lcol")
                    nc.sync.dma_start(out=lcol, in_=rowd)
                    with nc.allow_low_precision(reason="1/l fits fp16"):
                        nc.vector.reciprocal(out=rt, in_=lcol)
                tail = hp == 1 and c == NCH - 1
                deng = nc.scalar if tail else nc.sync
                rtd = dp.tile([128, 8], F16, tag="rtd")
                deng.dma_start(out=rtd, in_=rt)
                rbh = nrp.tile([64, 1024], F16, tag="rbh")
                deng.dma_start(
                    out=rbh,
                    in_=rtd.rearrange("p i -> (p i)")[None, :].to_broadcast([64, 1024]),
                )
                nc.vector.tensor_mul(
                    out=avt[0:64, ccols], in0=av[0:64, 0:512], in1=rbh[:, 0:512]
                )
                avh = nrp.tile([64, 512], F16, tag="avh")
                nc.vector.tensor_mul(
                    out=avh, in0=av[0:64, 512:1024], in1=rbh[:, 512:1024]
                )
                deng.dma_start(out=avt[64:128, ccols], in_=avh)

            def outproj_fillers(c):
                def mk(tt):
                    def f():
                        outproj_tile(c, tt)
                    return f
                return [mk(tt) for tt in range(4)]

            def outproj(c):
                # output projection for chunk c's 4 row tiles
                for tt in range(4):
                    outproj_tile(c, tt)

            def outproj_tile(c, tt):
                    trow = c * 4 + tt
                    tcols = slice(trow * 128, (trow + 1) * 128)
                    osb = op.tile([128, DIM], F16, tag="osb", name="osb")
                    for dch in range(2):
                        dcols = slice(dch * 512, (dch + 1) * 512)
                        po = pss.tile([128, 512], F32, tag="s")
                        nc.tensor.matmul(
                            po,
                            lhsT=avt01[:, tcols],
                            rhs=wo_sb[:, 0, dcols],
                            start=True,
                            stop=False,
                        )
                        nc.tensor.matmul(
                            po,
                            lhsT=avt23[:, tcols],
                            rhs=wo_sb[:, 1, dcols],
                            start=False,
                            stop=True,
                        )
                        cp(osb[:, dcols], po)
                    oeng = nc.sync if tt % 2 == 0 else nc.scalar
                    oeng.dma_start(
                        out=out[trow * 128 : (trow + 1) * 128, :], in_=osb
                    )

            # interleave: qkv of chunk c+1 and out-proj of chunk c-1 are
            # emitted between the head-pair groups of chunk c so PE always
            # has independent work while ACT drains the exp queue.
            qkv_chunk(0)
            for c in range(NCH):
                if c + 1 < NCH:
                    qkv_chunk(c + 1)
                attn_hp(0, c)
                # outproj(c-1) rides inside hp1(c) as per-row-tile fillers:
                # by then its avt inputs (previous chunk's normalize chain,
                # DMA-latency bound) are long complete, and the short
                # 4-matmul groups slot into the ACT-paced PE bubbles.
                attn_hp(1, c, outproj_fillers(c - 1) if c >= 1 else ())
            outproj(NCH - 1)

    nc.compile()
    return nc


def make_in_maps(x, Wq, Wkv, Wo):
    x = np.asarray(x, dtype=np.float32)
    Wq = np.asarray(Wq, dtype=np.float32)
    Wkv = np.asarray(Wkv, dtype=np.float32)
    Wo = np.asarray(Wo, dtype=np.float32)
    idx = np.arange(128)
    ltri_np = (idx[:, None] <= idx[None, :]).astype(np.float16)  # key <= query
    idh_np = np.eye(128, dtype=np.float16)
    in_maps = []
    for core in range(8):
        b, g = divmod(core, NKV)
        k_loc = Wkv[:, g * HD : (g + 1) * HD]
        v_loc = Wkv[:, NKV * HD + g * HD : NKV * HD + (g + 1) * HD]
        in_maps.append(
            {
                "xT": np.ascontiguousarray(x[b].T).astype(np.float16),
                "wq": np.ascontiguousarray(
                    Wq[:, g * HQ : (g + 1) * HQ]
                ).astype(np.float16),
                "wkv": np.ascontiguousarray(
                    np.concatenate([k_loc, v_loc], axis=1)
                ).astype(np.float16),
                "wo": np.ascontiguousarray(Wo[g * HQ : (g + 1) * HQ, :]).astype(
                    np.float16
                ),
                "ltri": ltri_np,
                "idh": idh_np,
            }
        )
    return in_maps


def gather(results):
    outs = [results[i]["out"].astype(np.float64) for i in range(8)]
    return np.stack(
        [
            outs[0] + outs[1] + outs[2] + outs[3],
            outs[4] + outs[5] + outs[6] + outs[7],
        ]
    ).astype(np.float32)


def kernel(x, Wq, Wkv, Wo):
    global _CACHED_NC
    if _CACHED_NC is None:
        _CACHED_NC = build_nc()
    in_maps = make_in_maps(x, Wq, Wkv, Wo)
    res = run_bass_kernel_spmd(_CACHED_NC, in_maps, list(range(8)))
    return gather(res.results)


# revision 37
# speedup vs baseline: 1.0195x; 1.0195x over previous
"""GroupedQueryAttention TRN2 kernel (v2).

Sharding: 8 cores = (batch b in 0..1) x (kv-group g in 0..3). Each core
computes, for its batch and its kv head group (1 kv head, 4 query heads):
  q = x[b] @ Wq[:, g*256:(g+1)*256]          [2048, 256]
  k = x[b] @ Wkv[:, g*64:(g+1)*64]           [2048, 64]
  v = x[b] @ Wkv[:, 256+g*64:256+(g+1)*64]   [2048, 64]
  causal softmax attention per head          [2048, 256]
  partial_out = attn_out @ Wo[g*256:(g+1)*256, :]   [2048, 1024]
Host sums the 4 partials per batch (row-parallel Wo).

All operands fp16 (same precision class as fp32r, half the weight-load /
DVE / DMA cost). On-chip layout is fully transposed (feature dims on
partitions):
  - scores are computed as S^T[tk, tq]; heads are processed in parity
    pairs (even head kT/qT at partitions 0:64, odd at 64:128) so the two
    64-row score matmuls land on different PE row groups and overlap.
  - both heads' scores for one key tile share a psum tile (even slab at
    bank 0, odd at bank 1) -> usually one exp per key tile.
  - causal masking: exp everything, then zero above-diagonal
    probabilities with a 0/1 fp16 mask multiply on the (otherwise idle)
    Pool engine.
  - softmax denominators come from ones-columns appended to v. The
    normalization reshapes l to partition-major via a DRAM bounce (a
    [1,1024] single-lane DVE reciprocal costs ~6.5us on HW; [128,8] is
    ~60ns), reciprocals there, and partition-broadcasts 1/l back from
    DRAM; queries stay on the free axis throughout (no transposes).
    The very last group uses PE transposes instead of the DRAM bounce
    (the PE is idle on the tail and DMA latency is on the critical path).
  - odd-parity avT slabs reach avt partitions 64:128 via SBUF->SBUF DMA;
    kT for the odd parity is duplicated to partitions 64:128 by DMA.
  - DMA issue costs ~600ns on the issuing engine; startup/tail issues
    are split between the two HWDGE engines (SP + ACT).
  - out-projection of chunk c-1 is emitted after both head pairs of
    chunk c so its PE work covers the DMA-latency-bound normalize chain.
"""

import numpy as np
import ml_dtypes

import concourse.bass as bass
import concourse.mybir as mybir
import concourse.tile as tile
from concourse import bacc
from concourse.bass_utils import run_bass_kernel_spmd

B, T, DIM = 2, 2048, 1024
NH, NKV = 16, 4
HD = DIM // NH  # 64
R = NH // NKV  # 4
HQ = R * HD  # 256 query cols per core
NJ = T // 128  # 16 key tiles
NCH = T // 512  # 4 query chunks of 512

F16 = mybir.dt.float16
F32 = mybir.dt.float32
F32R = mybir.dt.float32r
F8 = mybir.dt.float8e4

_CACHED_NC = None


def _cfg(c, j):
    """Per (chunk, key-tile): (tq start within chunk, width)."""
    m = j - 4 * c
    if m < 0:
        return 0, 512
    return 128 * m, 512 - 128 * m


def build_nc():
    nc = bacc.Bacc()
    xT = nc.declare_dram_parameter("xT", [DIM, T], F16, isOutput=False)
    wq = nc.declare_dram_parameter("wq", [DIM, HQ], F16, isOutput=False)
    wkv = nc.declare_dram_parameter("wkv", [DIM, 128], F16, isOutput=False)
    wo = nc.declare_dram_parameter("wo", [HQ, DIM], F16, isOutput=False)
    ltri = nc.declare_dram_parameter("ltri", [128, 128], F16, isOutput=False)
    idh = nc.declare_dram_parameter("idh", [128, 128], F16, isOutput=False)
    out = nc.declare_dram_parameter("out", [T, DIM], F16, isOutput=True)

    with tile.TileContext(nc) as tc:
        with (
            tc.tile_pool(name="persist", bufs=1) as pp,
            tc.tile_pool(name="vaug_p", bufs=NJ) as vp,
            tc.tile_pool(name="pt_p", bufs=4) as ptp,
            tc.tile_pool(name="nrm_p", bufs=3) as nrp,
            tc.tile_pool(name="out_p", bufs=3) as op,
            tc.tile_pool(name="dram_p", bufs=2, space="DRAM") as dp,
            tc.tile_pool(name="ps_s", bufs=2, space="PSUM") as pss,
            tc.tile_pool(name="ps_av", bufs=2, space="PSUM") as psav,
        ):
            # ---- constants / weights ----
            # DMA issue costs ~600ns each on the issuing engine; split the
            # startup issues between the two HWDGE engines (SP + ACT, which
            # is idle until the first exp) so the first qkv matmul is fed
            # as early as possible.
            wq_sb = pp.tile([128, 8, HQ], F16, tag="wq")
            nc.scalar.dma_start(out=wq_sb, in_=wq.rearrange("(k p) m -> p k m", p=128))
            xt_sb = pp.tile([128, 8, T], F16, tag="xt")
            for kd in range(8):
                deng = nc.sync if kd % 2 == 0 else nc.scalar
                deng.dma_start(
                    out=xt_sb[:, kd, 0:512],
                    in_=xT[kd * 128 : (kd + 1) * 128, 0:512],
                )
            wkv_sb = pp.tile([128, 8, 128], F16, tag="wkv")
            nc.scalar.dma_start(
                out=wkv_sb, in_=wkv.rearrange("(k p) m -> p k m", p=128)
            )
            ident = pp.tile([128, 128], F16, tag="ident")
            nc.sync.dma_start(out=ident, in_=idh[:, :])
            idf32 = pp.tile([1, 1], F32, tag="idf32")
            nc.gpsimd.memset(idf32, 1.0)
            ltri_sb = pp.tile([128, 128], F16, tag="ltri")
            nc.sync.dma_start(out=ltri_sb, in_=ltri[:, :])
            xT_r = xT.rearrange("(k p) t -> p k t", p=128)
            for th in range(1, 4):
                tc_cols = slice(th * 512, (th + 1) * 512)
                deng = nc.sync if th % 2 == 1 else nc.scalar
                deng.dma_start(out=xt_sb[:, :, tc_cols], in_=xT_r[:, :, tc_cols])
            wo_sb = pp.tile([128, 2, DIM], F16, tag="wo")
            nc.sync.dma_start(out=wo_sb, in_=wo.rearrange("(c p) n -> p c n", p=128))

            qt_sb = pp.tile([128, 2, T], F16, tag="qt")  # heads (2h, 2h+1) pairs
            kv_sb = pp.tile([128, T], F16, tag="kv")  # 0:64 kT, 64:128 vT
            kvb_sb = pp.tile([128, T], F16, tag="kvb")  # 64:128 kT dup
            avt01 = pp.tile([128, T], F16, tag="avt01")
            avt23 = pp.tile([128, T], F16, tag="avt23")
            eng = [0]

            def cp(dst, src):
                # alternate drain engine to balance ACT/DVE load
                if eng[0] % 2 == 0:
                    nc.scalar.copy(dst, src)
                else:
                    nc.vector.tensor_copy(out=dst, in_=src)
                eng[0] += 1

            # ---- qkv projections (xT stationary, weights as lhsT) ----
            vaug = [None] * NJ

            def qkv_chunk(n):
                cols = slice(n * 512, (n + 1) * 512)
                for m in range(2):
                    pq = pss.tile([128, 512], F32, tag="s")
                    for kd in range(8):
                        nc.tensor.matmul(
                            pq,
                            lhsT=wq_sb[:, kd, m * 128 : (m + 1) * 128],
                            rhs=xt_sb[:, kd, cols],
                            start=(kd == 0),
                            stop=(kd == 7),
                        )
                    cp(qt_sb[:, m, cols], pq)
                pkv = pss.tile([128, 512], F32, tag="s")
                for kd in range(8):
                    nc.tensor.matmul(
                        pkv,
                        lhsT=wkv_sb[:, kd, :],
                        rhs=xt_sb[:, kd, cols],
                        start=(kd == 0),
                        stop=(kd == 7),
                    )
                cp(kv_sb[0:64, cols], pkv[0:64, :])
                cp(kv_sb[64:128, cols], pkv[64:128, :])
                # kT dup for odd heads via SBUF->SBUF DMA
                nc.sync.dma_start(out=kvb_sb[64:128, cols], in_=kv_sb[0:64, cols])
                for tt in range(4):
                    j = n * 4 + tt
                    # vT -> v via XBAR DMA transpose (SBUF->SBUF): no PE
                    # transpose, no psum tile, no ACT drain copy.
                    va = vp.tile([128, 66], F16, tag="vaug")
                    deng = nc.sync if tt % 2 == 0 else nc.scalar
                    deng.dma_start_transpose(
                        va[:, 0:64], kv_sb[64:128, j * 128 : (j + 1) * 128]
                    )
                    nc.gpsimd.memset(va[:, 64:66], 1.0)
                    vaug[j] = va

            # ---- attention: one parity-pair of heads over chunk c ----
            def attn_hp(hp, c):
                """hp in {0,1}: heads (2hp, 2hp+1). Even head at partitions
                0:64, odd at 64:128; their score matmuls overlap on
                different PE row groups."""
                jmax = 4 * c + 3
                avt = avt01 if hp == 0 else avt23
                av = psav.tile([66, 1024], F32, tag="av")
                for j in range(jmax + 1):
                    sa, w = _cfg(c, j)
                    spt = pss.tile([128, 1024], F32, tag="s")
                    ptt = ptp.tile([128, 1024], F16, tag="pt")
                    jc = slice(j * 128, (j + 1) * 128)
                    qc = slice(c * 512 + sa, c * 512 + sa + w)
                    # even head at psum cols 0:w (bank 0), odd head at
                    # 512:512+w (bank 1) — matmul outputs may not cross a
                    # psum bank boundary.
                    nc.tensor.matmul(
                        spt[:, 0:w],
                        lhsT=kv_sb[0:64, jc],
                        rhs=qt_sb[0:64, hp, qc],
                        start=True,
                        stop=True,
                    )
                    nc.tensor.matmul(
                        spt[:, 512 : 512 + w],
                        lhsT=kvb_sb[64:128, jc],
                        rhs=qt_sb[64:128, hp, qc],
                        start=True,
                        stop=True,
                    )
                    if w == 512:
                        nc.scalar.activation(
                            out=ptt[:, 0:1024],
                            in_=spt[:, 0:1024],
                            func=mybir.ActivationFunctionType.Exp,
                            scale=0.125,
                        )
                    else:
                        nc.scalar.activation(
                            out=ptt[:, 0:w],
                            in_=spt[:, 0:w],
                            func=mybir.ActivationFunctionType.Exp,
                            scale=0.125,
                        )
                        nc.scalar.activation(
                            out=ptt[:, 512 : 512 + w],
                            in_=spt[:, 512 : 512 + w],
                            func=mybir.ActivationFunctionType.Exp,
                            scale=0.125,
                        )
                    if j >= 4 * c:
                        # zero above-diagonal probs (tile-local cols 0:128
                        # of each head's slab)
                        nc.gpsimd.tensor_mul(
                            out=ptt[:, 0:128], in0=ptt[:, 0:128], in1=ltri_sb
                        )
                        nc.gpsimd.tensor_mul(
                            out=ptt[:, 512:640],
                            in0=ptt[:, 512:640],
                            in1=ltri_sb,
                        )
                    nc.tensor.matmul(
                        av[:, sa : sa + w],
                        lhsT=vaug[j][:, 0:66],
                        rhs=ptt[:, 0:w],
                        start=(j == 0),
                        stop=(j == jmax),
                    )
                    nc.tensor.matmul(
                        av[:, 512 + sa : 512 + sa + w],
                        lhsT=vaug[j][:, 0:66],
                        rhs=ptt[:, 512 : 512 + w],
                        start=(j == 0),
                        stop=(j == jmax),
                    )
                # normalize: avT[:, q] /= l[q]  (queries on the free axis).
                # A [1,1024] single-lane DVE reciprocal costs ~6.5us on HW, so
                # reshape l to partition-major (via a DRAM bounce — SBUF APs
                # cannot exchange partition/free dims), reciprocal there
                # (~60ns), then bounce back and partition-broadcast from DRAM.
                ccols = slice(c * 512, (c + 1) * 512)
                rt = nrp.tile([128, 8], F16, tag="rt")
                if hp == 1 and c == NCH - 1:
                    # tail group: the l->column reshape sits on the critical
                    # path with an otherwise-idle PE; transpose on the PE
                    # instead of the DRAM round trip (saves ~4us of latency).
                    # Strided fp32 slices keep the q = p*8+i layout of the
                    # DMA path and 4-byte weight/psum alignment.
                    lrow32 = nrp.tile([1, 1024], F32, tag="lrow32")
                    nc.vector.tensor_copy(out=lrow32, in_=av[64:65, :])
                    lrow_r = lrow32.rearrange("o (p i) -> o p i", p=128)
                    ptl = pss.tile([128, 8], F32, tag="s")
                    for i in range(8):
                        nc.tensor.transpose(
                            ptl[:, i : i + 1],
                            in_=lrow_r[0:1, :, i],
                            identity=idf32,
                        )
                    with nc.allow_low_precision(reason="1/l fits fp16"):
                        nc.vector.reciprocal(out=rt, in_=ptl)
                else:
                    lrow = nrp.tile([1, 1024], F16, tag="lrow")
                    with nc.allow_low_precision(reason="l fits fp16"):
                        nc.vector.tensor_copy(out=lrow, in_=av[64:65, :])
                    rowd = dp.tile([128, 8], F16, tag="rowd")
                    nc.sync.dma_start(out=rowd, in_=lrow)
                    lcol = nrp.tile([128, 8], F16, tag="lcol")
                    nc.sync.dma_start(out=lcol, in_=rowd)
                    with nc.allow_low_precision(reason="1/l fits fp16"):
                        nc.vector.reciprocal(out=rt, in_=lcol)
                tail = hp == 1 and c == NCH - 1
                deng = nc.scalar if tail else nc.sync
                rtd = dp.tile([128, 8], F16, tag="rtd")
                deng.dma_start(out=rtd, in_=rt)
                rbh = nrp.tile([64, 1024], F16, tag="rbh")
                deng.dma_start(
                    out=rbh,
                    in_=rtd.rearrange("p i -> (p i)")[None, :].to_broadcast([64, 1024]),
                )
                nc.vector.tensor_mul(
                    out=avt[0:64, ccols], in0=av[0:64, 0:512], in1=rbh[:, 0:512]
                )
                avh = nrp.tile([64, 512], F16, tag="avh")
                nc.vector.tensor_mul(
                    out=avh, in0=av[0:64, 512:1024], in1=rbh[:, 512:1024]
                )
                deng.dma_start(out=avt[64:128, ccols], in_=avh)

            def outproj(c):
                # output projection for chunk c's 4 row tiles
                for tt in range(4):
                    trow = c * 4 + tt
                    tcols = slice(trow * 128, (trow + 1) * 128)
                    osb = op.tile([128, DIM], F16, tag="osb")
                    for dch in range(2):
                        dcols = slice(dch * 512, (dch + 1) * 512)
                        po = pss.tile([128, 512], F32, tag="s")
                        nc.tensor.matmul(
                            po,
                            lhsT=avt01[:, tcols],
                            rhs=wo_sb[:, 0, dcols],
                            start=True,
                            stop=False,
                        )
                        nc.tensor.matmul(
                            po,
                            lhsT=avt23[:, tcols],
                            rhs=wo_sb[:, 1, dcols],
                            start=False,
                            stop=True,
                        )
                        cp(osb[:, dcols], po)
                    nc.sync.dma_start(
                        out=out[trow * 128 : (trow + 1) * 128, :], in_=osb
                    )

            # interleave: qkv of chunk c+1 and out-proj of chunk c-1 are
            # emitted between the head-pair groups of chunk c so PE always
            # has independent work while ACT drains the exp queue.
            qkv_chunk(0)
            for c in range(NCH):
                if c + 1 < NCH:
                    qkv_chunk(c + 1)
                attn_hp(0, c)
                attn_hp(1, c)
                # outproj after hp1 so its PE work covers the normalize
                # chain (DMA-latency bound) of both head-pair groups
                if c >= 1:
                    outproj(c - 1)
            outproj(NCH - 1)

    nc.compile()
    return nc


def make_in_maps(x, Wq, Wkv, Wo):
    x = np.asarray(x, dtype=np.float32)
    Wq = np.asarray(Wq, dtype=np.float32)
    Wkv = np.asarray(Wkv, dtype=np.float32)
    Wo = np.asarray(Wo, dtype=np.float32)
    idx = np.arange(128)
    ltri_np = (idx[:, None] <= idx[None, :]).astype(np.float16)  # key <= query
    idh_np = np.eye(128, dtype=np.float16)
    in_maps = []
    for core in range(8):
        b, g = divmod(core, NKV)
        k_loc = Wkv[:, g * HD : (g + 1) * HD]
        v_loc = Wkv[:, NKV * HD + g * HD : NKV * HD + (g + 1) * HD]
        in_maps.append(
            {
                "xT": np.ascontiguousarray(x[b].T).astype(np.float16),
                "wq": np.ascontiguousarray(
                    Wq[:, g * HQ : (g + 1) * HQ]
                ).astype(np.float16),
                "wkv": np.ascontiguousarray(
                    np.concatenate([k_loc, v_loc], axis=1)
                ).astype(np.float16),
                "wo": np.ascontiguousarray(Wo[g * HQ : (g + 1) * HQ, :]).astype(
                    np.float16
                ),
                "ltri": ltri_np,
                "idh": idh_np,
            }
        )
    return in_maps


def gather(results):
    outs = [results[i]["out"].astype(np.float64) for i in range(8)]
    return np.stack(
        [
            outs[0] + outs[1] + outs[2] + outs[3],
            outs[4] + outs[5] + outs[6] + outs[7],
        ]
    ).astype(np.float32)


def kernel(x, Wq, Wkv, Wo):
    global _CACHED_NC
    if _CACHED_NC is None:
        _CACHED_NC = build_nc()
    in_maps = make_in_maps(x, Wq, Wkv, Wo)
    res = run_bass_kernel_spmd(_CACHED_NC, in_maps, list(range(8)))
    return gather(res.results)


# revision 38
# speedup vs baseline: 1.0599x; 1.0396x over previous
"""GroupedQueryAttention TRN2 kernel (v2).

Sharding: 8 cores = (batch b in 0..1) x (kv-group g in 0..3). Each core
computes, for its batch and its kv head group (1 kv head, 4 query heads):
  q = x[b] @ Wq[:, g*256:(g+1)*256]          [2048, 256]
  k = x[b] @ Wkv[:, g*64:(g+1)*64]           [2048, 64]
  v = x[b] @ Wkv[:, 256+g*64:256+(g+1)*64]   [2048, 64]
  causal softmax attention per head          [2048, 256]
  partial_out = attn_out @ Wo[g*256:(g+1)*256, :]   [2048, 1024]
Host sums the 4 partials per batch (row-parallel Wo).

All operands fp16 (same precision class as fp32r, half the weight-load /
DVE / DMA cost). On-chip layout is fully transposed (feature dims on
partitions):
  - scores are computed as S^T[tk, tq]; heads are processed in parity
    pairs (even head kT/qT at partitions 0:64, odd at 64:128) so the two
    64-row score matmuls land on different PE row groups and overlap.
  - both heads' scores for one key tile share a psum tile (even slab at
    bank 0, odd at bank 1) -> usually one exp per key tile.
  - causal masking: exp everything, then zero above-diagonal
    probabilities with a 0/1 fp16 mask multiply on the (otherwise idle)
    Pool engine.
  - softmax denominators come from ones-columns appended to v. The
    normalization reshapes l to partition-major via a DRAM bounce (a
    [1,1024] single-lane DVE reciprocal costs ~6.5us on HW; [128,8] is
    ~60ns), reciprocals there, and partition-broadcasts 1/l back from
    DRAM; queries stay on the free axis throughout (no transposes).
    The very last group uses PE transposes instead of the DRAM bounce
    (the PE is idle on the tail and DMA latency is on the critical path).
  - odd-parity avT slabs reach avt partitions 64:128 via SBUF->SBUF DMA;
    kT for the odd parity is duplicated to partitions 64:128 by DMA.
  - DMA issue costs ~600ns on the issuing engine; startup/tail issues
    are split between the two HWDGE engines (SP + ACT).
  - out-projection of chunk c-1 is emitted after both head pairs of
    chunk c so its PE work covers the DMA-latency-bound normalize chain.
"""

import numpy as np
import ml_dtypes

import concourse.bass as bass
import concourse.mybir as mybir
import concourse.tile as tile
from concourse import bacc
from concourse.bass_utils import run_bass_kernel_spmd

B, T, DIM = 2, 2048, 1024
NH, NKV = 16, 4
HD = DIM // NH  # 64
R = NH // NKV  # 4
HQ = R * HD  # 256 query cols per core
NJ = T // 128  # 16 key tiles
NCH = T // 512  # 4 query chunks of 512

F16 = mybir.dt.float16
F32 = mybir.dt.float32
F32R = mybir.dt.float32r
F8 = mybir.dt.float8e4

_CACHED_NC = None


def _cfg(c, j):
    """Per (chunk, key-tile): (tq start within chunk, width)."""
    m = j - 4 * c
    if m < 0:
        return 0, 512
    return 128 * m, 512 - 128 * m


def build_nc():
    nc = bacc.Bacc()
    xT = nc.declare_dram_parameter("xT", [DIM, T], F16, isOutput=False)
    wq = nc.declare_dram_parameter("wq", [DIM, HQ], F16, isOutput=False)
    wkv = nc.declare_dram_parameter("wkv", [DIM, 128], F16, isOutput=False)
    wo = nc.declare_dram_parameter("wo", [HQ, DIM], F16, isOutput=False)
    ltri = nc.declare_dram_parameter("ltri", [128, 128], F16, isOutput=False)
    idh = nc.declare_dram_parameter("idh", [128, 128], F16, isOutput=False)
    out = nc.declare_dram_parameter("out", [T, DIM], F16, isOutput=True)

    with tile.TileContext(nc) as tc:
        with (
            tc.tile_pool(name="persist", bufs=1) as pp,
            tc.tile_pool(name="vaug_p", bufs=NJ) as vp,
            tc.tile_pool(name="pt_p", bufs=4) as ptp,
            tc.tile_pool(name="nrm_p", bufs=3) as nrp,
            tc.tile_pool(name="out_p", bufs=3) as op,
            tc.tile_pool(name="dram_p", bufs=2, space="DRAM") as dp,
            tc.tile_pool(name="ps_s", bufs=2, space="PSUM") as pss,
            tc.tile_pool(name="ps_av", bufs=2, space="PSUM") as psav,
        ):
            # ---- constants / weights ----
            # DMA issue costs ~600ns each on the issuing engine; split the
            # startup issues between the two HWDGE engines (SP + ACT, which
            # is idle until the first exp) so the first qkv matmul is fed
            # as early as possible.
            wq_sb = pp.tile([128, 8, HQ], F16, tag="wq")
            nc.scalar.dma_start(out=wq_sb, in_=wq.rearrange("(k p) m -> p k m", p=128))
            xt_sb = pp.tile([128, 8, T], F16, tag="xt")
            for kd in range(8):
                deng = nc.sync if kd % 2 == 0 else nc.scalar
                deng.dma_start(
                    out=xt_sb[:, kd, 0:512],
                    in_=xT[kd * 128 : (kd + 1) * 128, 0:512],
                )
            wkv_sb = pp.tile([128, 8, 128], F16, tag="wkv")
            nc.scalar.dma_start(
                out=wkv_sb, in_=wkv.rearrange("(k p) m -> p k m", p=128)
            )
            ident = pp.tile([128, 128], F16, tag="ident")
            nc.sync.dma_start(out=ident, in_=idh[:, :])
            idf32 = pp.tile([1, 1], F32, tag="idf32")
            nc.gpsimd.memset(idf32, 1.0)
            ltri_sb = pp.tile([128, 128], F16, tag="ltri")
            nc.sync.dma_start(out=ltri_sb, in_=ltri[:, :])
            xT_r = xT.rearrange("(k p) t -> p k t", p=128)
            for th in range(1, 4):
                tc_cols = slice(th * 512, (th + 1) * 512)
                deng = nc.sync if th % 2 == 1 else nc.scalar
                deng.dma_start(out=xt_sb[:, :, tc_cols], in_=xT_r[:, :, tc_cols])
            wo_sb = pp.tile([128, 2, DIM], F16, tag="wo")
            nc.sync.dma_start(out=wo_sb, in_=wo.rearrange("(c p) n -> p c n", p=128))

            qt_sb = pp.tile([128, 2, T], F16, tag="qt")  # heads (2h, 2h+1) pairs
            kv_sb = pp.tile([128, T], F16, tag="kv")  # 0:64 kT, 64:128 vT
            kvb_sb = pp.tile([128, T], F16, tag="kvb")  # 64:128 kT dup
            avt01 = pp.tile([128, T], F16, tag="avt01")
            avt23 = pp.tile([128, T], F16, tag="avt23")
            eng = [0]

            def cp(dst, src):
                # alternate drain engine to balance ACT/DVE load
                if eng[0] % 2 == 0:
                    nc.scalar.copy(dst, src)
                else:
                    nc.vector.tensor_copy(out=dst, in_=src)
                eng[0] += 1

            # ---- qkv projections (xT stationary, weights as lhsT) ----
            vaug = [None] * NJ

            def qkv_chunk(n):
                cols = slice(n * 512, (n + 1) * 512)
                for m in range(2):
                    pq = pss.tile([128, 512], F32, tag="s")
                    for kd in range(8):
                        nc.tensor.matmul(
                            pq,
                            lhsT=wq_sb[:, kd, m * 128 : (m + 1) * 128],
                            rhs=xt_sb[:, kd, cols],
                            start=(kd == 0),
                            stop=(kd == 7),
                        )
                    cp(qt_sb[:, m, cols], pq)
                pkv = pss.tile([128, 512], F32, tag="s")
                for kd in range(8):
                    nc.tensor.matmul(
                        pkv,
                        lhsT=wkv_sb[:, kd, :],
                        rhs=xt_sb[:, kd, cols],
                        start=(kd == 0),
                        stop=(kd == 7),
                    )
                cp(kv_sb[0:64, cols], pkv[0:64, :])
                cp(kv_sb[64:128, cols], pkv[64:128, :])
                # kT dup for odd heads via SBUF->SBUF DMA
                nc.sync.dma_start(out=kvb_sb[64:128, cols], in_=kv_sb[0:64, cols])
                for tt in range(4):
                    j = n * 4 + tt
                    ptr = pss.tile([128, 64], F16, tag="s")
                    nc.tensor.transpose(
                        ptr,
                        in_=kv_sb[64:128, j * 128 : (j + 1) * 128],
                        identity=ident[64:128, 64:128],
                    )
                    va = vp.tile([128, 66], F16, tag="vaug")
                    nc.scalar.copy(va[:, 0:64], ptr)
                    nc.gpsimd.memset(va[:, 64:66], 1.0)
                    vaug[j] = va

            # ---- attention: one parity-pair of heads over chunk c ----
            def attn_hp(hp, c):
                """hp in {0,1}: heads (2hp, 2hp+1). Even head at partitions
                0:64, odd at 64:128; their score matmuls overlap on
                different PE row groups."""
                jmax = 4 * c + 3
                avt = avt01 if hp == 0 else avt23
                av = psav.tile([66, 1024], F32, tag="av")
                for j in range(jmax + 1):
                    sa, w = _cfg(c, j)
                    spt = pss.tile([128, 1024], F32, tag="s")
                    ptt = ptp.tile([128, 1024], F16, tag="pt")
                    jc = slice(j * 128, (j + 1) * 128)
                    qc = slice(c * 512 + sa, c * 512 + sa + w)
                    # even head at psum cols 0:w (bank 0), odd head at
                    # 512:512+w (bank 1) — matmul outputs may not cross a
                    # psum bank boundary.
                    nc.tensor.matmul(
                        spt[:, 0:w],
                        lhsT=kv_sb[0:64, jc],
                        rhs=qt_sb[0:64, hp, qc],
                        start=True,
                        stop=True,
                    )
                    nc.tensor.matmul(
                        spt[:, 512 : 512 + w],
                        lhsT=kvb_sb[64:128, jc],
                        rhs=qt_sb[64:128, hp, qc],
                        start=True,
                        stop=True,
                    )
                    if w == 512:
                        nc.scalar.activation(
                            out=ptt[:, 0:1024],
                            in_=spt[:, 0:1024],
                            func=mybir.ActivationFunctionType.Exp,
                            scale=0.125,
                        )
                    else:
                        nc.scalar.activation(
                            out=ptt[:, 0:w],
                            in_=spt[:, 0:w],
                            func=mybir.ActivationFunctionType.Exp,
                            scale=0.125,
                        )
                        nc.scalar.activation(
                            out=ptt[:, 512 : 512 + w],
                            in_=spt[:, 512 : 512 + w],
                            func=mybir.ActivationFunctionType.Exp,
                            scale=0.125,
                        )
                    if j >= 4 * c:
                        # zero above-diagonal probs (tile-local cols 0:128
                        # of each head's slab)
                        nc.gpsimd.tensor_mul(
                            out=ptt[:, 0:128], in0=ptt[:, 0:128], in1=ltri_sb
                        )
                        nc.gpsimd.tensor_mul(
                            out=ptt[:, 512:640],
                            in0=ptt[:, 512:640],
                            in1=ltri_sb,
                        )
                    nc.tensor.matmul(
                        av[:, sa : sa + w],
                        lhsT=vaug[j][:, 0:66],
                        rhs=ptt[:, 0:w],
                        start=(j == 0),
                        stop=(j == jmax),
                    )
                    nc.tensor.matmul(
                        av[:, 512 + sa : 512 + sa + w],
                        lhsT=vaug[j][:, 0:66],
                        rhs=ptt[:, 512 : 512 + w],
                        start=(j == 0),
                        stop=(j == jmax),
                    )
                # normalize: avT[:, q] /= l[q]  (queries on the free axis).
                # A [1,1024] single-lane DVE reciprocal costs ~6.5us on HW, so
                # reshape l to partition-major (via a DRAM bounce — SBUF APs
                # cannot exchange partition/free dims), reciprocal there
                # (~60ns), then bounce back and partition-broadcast from DRAM.
                ccols = slice(c * 512, (c + 1) * 512)
                rt = nrp.tile([128, 8], F16, tag="rt")
                if hp == 1 and c == NCH - 1:
                    # tail group: the l->column reshape sits on the critical
                    # path with an otherwise-idle PE; transpose on the PE
                    # instead of the DRAM round trip (saves ~4us of latency).
                    # Strided fp32 slices keep the q = p*8+i layout of the
                    # DMA path and 4-byte weight/psum alignment.
                    lrow32 = nrp.tile([1, 1024], F32, tag="lrow32")
                    nc.vector.tensor_copy(out=lrow32, in_=av[64:65, :])
                    lrow_r = lrow32.rearrange("o (p i) -> o p i", p=128)
                    ptl = pss.tile([128, 8], F32, tag="s")
                    for i in range(8):
                        nc.tensor.transpose(
                            ptl[:, i : i + 1],
                            in_=lrow_r[0:1, :, i],
                            identity=idf32,
                        )
                    with nc.allow_low_precision(reason="1/l fits fp16"):
                        nc.vector.reciprocal(out=rt, in_=ptl)
                else:
                    lrow = nrp.tile([1, 1024], F16, tag="lrow")
                    with nc.allow_low_precision(reason="l fits fp16"):
                        nc.vector.tensor_copy(out=lrow, in_=av[64:65, :])
                    rowd = dp.tile([128, 8], F16, tag="rowd")
                    nc.sync.dma_start(out=rowd, in_=lrow)
                    lcol = nrp.tile([128, 8], F16, tag="lcol")
                    nc.sync.dma_start(out=lcol, in_=rowd)
                    with nc.allow_low_precision(reason="1/l fits fp16"):
                        nc.vector.reciprocal(out=rt, in_=lcol)
                tail = hp == 1 and c == NCH - 1
                deng = nc.scalar if tail else nc.sync
                rtd = dp.tile([128, 8], F16, tag="rtd")
                deng.dma_start(out=rtd, in_=rt)
                rbh = nrp.tile([64, 1024], F16, tag="rbh")
                deng.dma_start(
                    out=rbh,
                    in_=rtd.rearrange("p i -> (p i)")[None, :].to_broadcast([64, 1024]),
                )
                nc.vector.tensor_mul(
                    out=avt[0:64, ccols], in0=av[0:64, 0:512], in1=rbh[:, 0:512]
                )
                avh = nrp.tile([64, 512], F16, tag="avh")
                nc.vector.tensor_mul(
                    out=avh, in0=av[0:64, 512:1024], in1=rbh[:, 512:1024]
                )
                deng.dma_start(out=avt[64:128, ccols], in_=avh)

            def outproj(c):
                # output projection for chunk c's 4 row tiles
                for tt in range(4):
                    trow = c * 4 + tt
                    tcols = slice(trow * 128, (trow + 1) * 128)
                    osb = op.tile([128, DIM], F16, tag="osb")
                    for dch in range(2):
                        dcols = slice(dch * 512, (dch + 1) * 512)
                        po = pss.tile([128, 512], F32, tag="s")
                        nc.tensor.matmul(
                            po,
                            lhsT=avt01[:, tcols],
                            rhs=wo_sb[:, 0, dcols],
                            start=True,
                            stop=False,
                        )
                        nc.tensor.matmul(
                            po,
                            lhsT=avt23[:, tcols],
                            rhs=wo_sb[:, 1, dcols],
                            start=False,
                            stop=True,
                        )
                        cp(osb[:, dcols], po)
                    nc.sync.dma_start(
                        out=out[trow * 128 : (trow + 1) * 128, :], in_=osb
                    )

            # interleave: qkv of chunk c+1 and out-proj of chunk c-1 are
            # emitted between the head-pair groups of chunk c so PE always
            # has independent work while ACT drains the exp queue.
            qkv_chunk(0)
            for c in range(NCH):
                if c + 1 < NCH:
                    qkv_chunk(c + 1)
                attn_hp(0, c)
                attn_hp(1, c)
                # outproj after hp1 so its PE work covers the normalize
                # chain (DMA-latency bound) of both head-pair groups
                if c >= 1:
                    outproj(c - 1)
            outproj(NCH - 1)

    nc.compile()
    return nc


def make_in_maps(x, Wq, Wkv, Wo):
    x = np.asarray(x, dtype=np.float32)
    Wq = np.asarray(Wq, dtype=np.float32)
    Wkv = np.asarray(Wkv, dtype=np.float32)
    Wo = np.asarray(Wo, dtype=np.float32)
    idx = np.arange(128)
    ltri_np = (idx[:, None] <= idx[None, :]).astype(np.float16)  # key <= query
    idh_np = np.eye(128, dtype=np.float16)
    in_maps = []
    for core in range(8):
        b, g = divmod(core, NKV)
        k_loc = Wkv[:, g * HD : (g + 1) * HD]
        v_loc = Wkv[:, NKV * HD + g * HD : NKV * HD + (g + 1) * HD]
        in_maps.append(
            {
                "xT": np.ascontiguousarray(x[b].T).astype(np.float16),
                "wq": np.ascontiguousarray(
                    Wq[:, g * HQ : (g + 1) * HQ]
                ).astype(np.float16),
                "wkv": np.ascontiguousarray(
                    np.concatenate([k_loc, v_loc], axis=1)
                ).astype(np.float16),
                "wo": np.ascontiguousarray(Wo[g * HQ : (g + 1) * HQ, :]).astype(
                    np.float16
                ),
                "ltri": ltri_np,
                "idh": idh_np,
            }
        )
    return in_maps


def gather(results):
    outs = [results[i]["out"].astype(np.float64) for i in range(8)]
    return np.stack(
        [
            outs[0] + outs[1] + outs[2] + outs[3],
            outs[4] + outs[5] + outs[6] + outs[7],
        ]
    ).astype(np.float32)


def kernel(x, Wq, Wkv, Wo):
    global _CACHED_NC
    if _CACHED_NC is None:
        _CACHED_NC = build_nc()
    in_maps = make_in_maps(x, Wq, Wkv, Wo)
    res = run_bass_kernel_spmd(_CACHED_NC, in_maps, list(range(8)))
    return gather(res.results)
